# revision 11
# baseline (speedup 1.0000x reference)
"""Multi-head causal+padded attention on 8 TRN2 NeuronCores.

Strategy: data-parallel over batch (8 batches -> 8 cores, no collectives).
Per core, everything is computed in a transposed layout so that no PE
transposes of the attention matrix are needed:

  QT[h] = (q Wq^T)^T slice  [e=128, tq]     KT[h] likewise
  V[kc] = (k Wv^T) row-chunk [tk=128, he]   (natural layout)
  S^T(kc,:) = KT[h][:,kc]^T-block matmuls   [tk-part, tq-free]
  A^T = exp(s * S^T + mk_bias)              (pad mask folded into exp bias,
                                             causal diag masked by a bf16
                                             identity x (-1e30 tri) matmul
                                             injected into PSUM)
  row sums  = ones^T @ A^T  (replicated across 128 partitions by all-ones lhsT)
  outT[h]   = sum_kc V[kc,h]^T-block @ A^T
  attnT[h]  = outT[h] * recip(sums)
  out^T     = sum_h WuT[h]^T @ attnT[h] + corr + bu

Degenerate softmax rows (all keys masked / no causal-visible key) are fixed
up exactly via two per-head mean-of-V vectors folded through the output
projection as a rank-2 correction (host-computed 0/1 row selectors).
Matmuls run as float32r (full PE rate at N>=256, near-fp32 precision).
"""

import numpy as np
import ml_dtypes

import concourse.bacc as bacc
import concourse.mybir as mybir
import concourse.tile as tile
from concourse.bass_utils import run_bass_kernel_spmd

F32 = mybir.dt.float32
F32R = mybir.dt.float32r
BF16 = mybir.dt.bfloat16
F16 = mybir.dt.float16

import os
SCORE_DT = {"f32r": F32R, "f16": F16}[os.environ.get("K_SCORE_DT", "f32r")]
SOFT_DT = {"f32r": F32R, "f16": F16}[os.environ.get("K_SOFT_DT", "f16")]
TRI_NEG = -60000.0 if SOFT_DT == F16 else -1.0e30
_SOFT_NP = {F16: "float16", F32R: "float32"}

B, TQ, TK, E, H = 8, 1024, 1024, 128, 8
HE = H * E
SCALE = float(E) ** -0.5
NEG = -1.0e30


def _chunks(kc):
    """Absolute column ranges for score row kc, split at the 512 PSUM bank."""
    lo = kc * 128
    if lo < 512:
        return [(lo, 512), (512, TQ)]
    return [(lo, TQ)]


def _build():
    nc = bacc.Bacc("TRN2", target_bir_lowering=False, debug=False)
    dp = nc.declare_dram_parameter
    d_qT = dp("qT", [E, TQ], F32R, isOutput=False)
    d_kT = dp("kT", [E, TK], F32R, isOutput=False)
    d_wqT = dp("wqT", [E, HE], F32R, isOutput=False)
    d_wkT = dp("wkT", [E, HE], F32R, isOutput=False)
    d_wvT = dp("wvT", [E, HE], F32R, isOutput=False)
    d_wuT = dp("wuT", [HE, E], SOFT_DT, isOutput=False)
    d_mkb = dp("mkbias", [128, 8], F32, isOutput=False)
    d_tri = dp("trineg", [128, 128], SOFT_DT, isOutput=False)
    d_idb = dp("identb", [128, 128], SOFT_DT, isOutput=False)
    d_case = dp("caserow", [1, TQ], F32R, isOutput=False)
    d_brow = dp("brows", [2, TQ], F32R, isOutput=False)
    d_w2 = dp("w2", [2, E], F32R, isOutput=False)
    d_ones1 = dp("onesk1", [1, 128], F32R, isOutput=False)
    d_ones = dp("ones128", [128, 128], SOFT_DT, isOutput=False)
    d_bu = dp("bu", [E, 1], F32, isOutput=False)
    d_out = dp("out", [E, TQ], F32, isOutput=True)

    Exp = mybir.ActivationFunctionType.Exp
    Ident = mybir.ActivationFunctionType.Identity
    mult = mybir.AluOpType.mult
    mm = nc.tensor.matmul

    with tile.TileContext(nc) as tc:
        with (
            tc.tile_pool(name="const", bufs=1) as cp,
            tc.tile_pool(name="persist", bufs=1) as pp,
        ):
            # ---- constants ----
            wu = []
            for h in range(H):
                t = cp.tile([128, 128], SOFT_DT, tag=f"wu{h}", name=f"wu{h}")
                nc.sync.dma_start(out=t[:], in_=d_wuT[h * 128 : (h + 1) * 128, :])
                wu.append(t)
            mkb = cp.tile([128, 8], F32, tag="mkb", name="mkb")
            nc.sync.dma_start(out=mkb[:], in_=d_mkb[:])
            tri = cp.tile([128, 128], SOFT_DT, tag="tri", name="tri")
            nc.sync.dma_start(out=tri[:], in_=d_tri[:])
            idb = cp.tile([128, 128], SOFT_DT, tag="idb", name="idb")
            nc.sync.dma_start(out=idb[:], in_=d_idb[:])
            case = cp.tile([1, TQ], F32R, tag="case", name="case")
            nc.sync.dma_start(out=case[:], in_=d_case[:])
            brow = cp.tile([2, TQ], F32R, tag="brow", name="brow")
            nc.sync.dma_start(out=brow[:], in_=d_brow[:])
            ones1 = cp.tile([1, 128], F32R, tag="ones1", name="ones1")
            nc.sync.dma_start(out=ones1[:], in_=d_ones1[:])
            ones = cp.tile([128, 128], SOFT_DT, tag="ones", name="ones")
            nc.sync.dma_start(out=ones[:], in_=d_ones[:])
            bu = cp.tile([E, 1], F32, tag="bu", name="bu")
            nc.sync.dma_start(out=bu[:], in_=d_bu[:])
            w2 = cp.tile([2, 128], F32R, tag="w2", name="w2")
            nc.sync.dma_start(out=w2[:], in_=d_w2[:])

            # ---- persistent activations ----
            QT = [pp.tile([128, TQ], SCORE_DT, tag=f"QT{h}", name=f"QT{h}") for h in range(H)]
            KT = [pp.tile([128, TK], SCORE_DT, tag=f"KT{h}", name=f"KT{h}") for h in range(H)]
            V = [pp.tile([128, HE], SOFT_DT, tag=f"V{kc}", name=f"V{kc}") for kc in range(8)]
            attnT = [
                pp.tile([128, TQ], SOFT_DT, tag=f"attnT{h}", name=f"attnT{h}")
                for h in range(H)
            ]

            # ---- phase 1: projections ----
            with (
                tc.tile_pool(name="proj", bufs=1) as jp,
                tc.tile_pool(name="ppsum", bufs=3, space="PSUM") as jps,
            ):
                qTs = jp.tile([E, TQ], F32R, tag="qTs", name="qTs")
                nc.sync.dma_start(out=qTs[:], in_=d_qT[:])
                kTs = jp.tile([E, TK], F32R, tag="kTs", name="kTs")
                nc.sync.dma_start(out=kTs[:], in_=d_kT[:])
                wq = jp.tile([E, HE], F32R, tag="wq", name="wq")
                nc.sync.dma_start(out=wq[:], in_=d_wqT[:])
                wk = jp.tile([E, HE], F32R, tag="wk", name="wk")
                nc.sync.dma_start(out=wk[:], in_=d_wkT[:])
                wv = jp.tile([E, HE], F32R, tag="wv", name="wv")
                nc.sync.dma_start(out=wv[:], in_=d_wvT[:])

                n_evac = 0

                def evac(dst, src):
                    nonlocal n_evac
                    if n_evac % 2 == 0:
                        nc.vector.tensor_copy(dst, src)
                    else:
                        nc.scalar.copy(dst, src)
                    n_evac += 1

                for h in range(H):
                    ps = jps.tile([128, TQ], F32, tag="pps", name=f"psq{h}")
                    for a, b in ((0, 512), (512, TQ)):
                        mm(ps[:, a:b], wq[:, h * 128 : (h + 1) * 128],
                           qTs[:, a:b], start=True, stop=True)
                    evac(QT[h][:], ps[:])
                for h in range(H):
                    ps = jps.tile([128, TK], F32, tag="pps", name=f"psk{h}")
                    for a, b in ((0, 512), (512, TK)):
                        mm(ps[:, a:b], wk[:, h * 128 : (h + 1) * 128],
                           kTs[:, a:b], start=True, stop=True)
                    evac(KT[h][:], ps[:])
                for kc in range(8):
                    ps = jps.tile([128, HE], F32, tag="pps", name=f"psv{kc}")
                    for a, b in ((0, 512), (512, HE)):
                        mm(ps[:, a:b], kTs[:, kc * 128 : (kc + 1) * 128],
                           wv[:, a:b], start=True, stop=True)
                    evac(V[kc][:], ps[:])


            # ---- phase 3: attention, unit = (head, 512-col query half) ----
            with (
                tc.tile_pool(name="stps", bufs=3, space="PSUM") as sp,
                tc.tile_pool(name="accps", bufs=2, space="PSUM") as ap_,
                tc.tile_pool(name="atp", bufs=4) as atp,
                tc.tile_pool(name="ssp", bufs=2) as ssp,
            ):
                units = [(h, half) for h in range(H) for half in (0, 1)]
                state = {}

                def emit_epilogue(u):
                    h, half = units[u]
                    q0 = half * 512
                    sum_ps, out_ps = state[u]
                    sum_sb = ssp.tile([128, 512], F32, tag="sum_sb",
                                      name=f"ssb{u}")
                    nc.scalar.copy(sum_sb[:], sum_ps[:])
                    rb = ssp.tile([128, 512], F32, tag="rb", name=f"rb{u}")
                    nc.vector.reciprocal(out=rb[:], in_=sum_sb[:])
                    nc.vector.tensor_tensor(
                        out=attnT[h][:, q0 : q0 + 512], in0=out_ps[:],
                        in1=rb[:], op=mult,
                    )

                for u, (h, half) in enumerate(units):
                    q0 = half * 512
                    kcs = range(4) if half == 0 else range(8)
                    klast = kcs[-1]
                    sum_ps = ap_.tile([128, 512], F32, tag="sum_ps", name=f"sum{u}")
                    out_ps = ap_.tile([128, 512], F32, tag="out_ps", name=f"out{u}")
                    state[u] = (sum_ps, out_ps)
                    ats = {}

                    def consume(kc):
                        r0 = max(kc * 128 - q0, 0)
                        n = 512 - r0
                        mm(sum_ps[:, r0:512], ones[:], ats[kc][:, 0:n],
                           start=(kc == 0), stop=False)
                        mm(out_ps[:, r0:512],
                           V[kc][:, h * 128 : (h + 1) * 128],
                           ats[kc][:, 0:n], start=(kc == 0),
                           stop=(kc == klast))

                    for kc in kcs:
                        r0 = max(kc * 128 - q0, 0)
                        n = 512 - r0
                        diag = q0 <= kc * 128 < q0 + 512
                        st = sp.tile([128, 512], F32, tag="st", name=f"st{u}_{kc}")
                        mm(st[:, r0:512], KT[h][:, kc * 128 : (kc + 1) * 128],
                           QT[h][:, q0 + r0 : q0 + 512], start=True,
                           stop=not diag)
                        if diag:
                            mm(st[:, r0 : r0 + 128], idb[:], tri[:],
                               start=False, stop=True)
                        at = atp.tile([128, 512], SOFT_DT, tag="at", name=f"at{u}_{kc}")
                        ats[kc] = at
                        nc.scalar.activation(
                            out=at[:, 0:n], in_=st[:, r0:512], func=Exp,
                            bias=mkb[:, kc : kc + 1], scale=SCALE,
                        )
                        if kc >= 1:
                            consume(kc - 1)
                        if kc == 1 and u >= 1:
                            emit_epilogue(u - 1)
                    consume(klast)
                    mm(sum_ps[:], ones1[:], case[:, q0 : q0 + 512],
                       start=False, stop=True)
                emit_epilogue(len(units) - 1)

            # ---- phase 4: output projection ----
            with tc.tile_pool(name="finps", bufs=1, space="PSUM") as fp:
                fin = fp.tile([128, TQ], F32, tag="fin", name="fin")
                for h in range(H):
                    for a, b in ((0, 512), (512, TQ)):
                        mm(fin[:, a:b], wu[h][:], attnT[h][:, a:b],
                           start=(h == 0), stop=False)
                for a, b in ((0, 512), (512, TQ)):
                    mm(fin[:, a:b], w2[:], brow[:, a:b],
                       start=False, stop=True)
                outsb = pp.tile([E, TQ], F32, tag="outsb", name="outsb")
                nc.scalar.activation(
                    out=outsb[:], in_=fin[:], func=Ident, bias=bu[:, 0:1], scale=1.0
                )
                nc.sync.dma_start(out=d_out[:], in_=outsb[:])

    nc.compile()
    return nc


_NC = None


def _get_nc():
    global _NC
    if _NC is None:
        _NC = _build()
    return _NC


def _host_prep(q, k, mask_q, mask_k, Wq, Wk, Wv, Wu, bu):
    shared = {
        "wqT": np.ascontiguousarray(Wq.T),
        "wkT": np.ascontiguousarray(Wk.T),
        "wvT": np.ascontiguousarray(Wv.T),
        "wuT": np.ascontiguousarray(Wu.T).astype(_SOFT_NP[SOFT_DT]),
        "trineg": (TRI_NEG * np.tril(np.ones((128, 128), np.float32), -1)).astype(_SOFT_NP[SOFT_DT]),
        "identb": np.eye(128).astype(_SOFT_NP[SOFT_DT]),
        "onesk1": np.ones((1, 128), np.float32),
        "ones128": np.ones((128, 128)).astype(_SOFT_NP[SOFT_DT]),
        "bu": np.ascontiguousarray(bu[:, None]),
    }
    WuWv = (Wu @ Wv).astype(np.float32)
    in_maps = []
    for b in range(B):
        mq = mask_q[b, :, 0].astype(np.float32)
        mk = mask_k[b, :, 0].astype(np.float32)
        c01 = (np.cumsum(mk) >= 1.0).astype(np.float32)
        caseA = mq * c01
        b1 = mq * (1.0 - c01)
        b2 = 1.0 - mq
        s1m = 1.0 - mk
        denom = max(float(s1m.sum()), 1.0)
        wvecs = np.stack([s1m / denom, np.full(TK, 1.0 / TK, np.float32)], axis=1)
        w2 = (wvecs.T.astype(np.float32) @ k[b]) @ WuWv.T
        m = dict(shared)
        m["qT"] = np.ascontiguousarray(q[b].T)
        m["kT"] = np.ascontiguousarray(k[b].T)
        m["mkbias"] = np.ascontiguousarray(
            ((mk - 1.0) * -NEG).reshape(8, 128).T
        ).astype(np.float32)
        m["caserow"] = ((1.0 - caseA) * -NEG)[None, :].astype(np.float32)
        m["brows"] = np.stack([b1, b2]).astype(np.float32)
        m["w2"] = np.ascontiguousarray(w2.astype(np.float32))
        in_maps.append(m)
    return in_maps


def kernel(q, k, mask_q, mask_k, Wq, Wk, Wv, Wu, bu):
    nc = _get_nc()
    in_maps = _host_prep(q, k, mask_q, mask_k, Wq, Wk, Wv, Wu, bu)
    res = run_bass_kernel_spmd(nc, in_maps, list(range(B)))
    out = np.stack([np.ascontiguousarray(res.results[b]["out"].T) for b in range(B)])
    return out.astype(np.float32)


# revision 12
# speedup vs baseline: 1.1369x; 1.1369x over previous
"""Multi-head causal+padded attention on 8 TRN2 NeuronCores.

Strategy: data-parallel over batch (8 batches -> 8 cores, no collectives).
Per core, everything is computed in a transposed layout so that no PE
transposes of the attention matrix are needed:

  QT[h] = (q Wq^T)^T slice  [e=128, tq]     KT[h] likewise
  V[kc] = (k Wv^T) row-chunk [tk=128, he]   (natural layout)
  S^T(kc,:) = KT[h][:,kc]^T-block matmuls   [tk-part, tq-free]
  A^T = exp(s * S^T + mk_bias)              (pad mask folded into exp bias,
                                             causal diag masked by a bf16
                                             identity x (-1e30 tri) matmul
                                             injected into PSUM)
  row sums  = ones^T @ A^T  (replicated across 128 partitions by all-ones lhsT)
  outT[h]   = sum_kc V[kc,h]^T-block @ A^T
  attnT[h]  = outT[h] * recip(sums)
  out^T     = sum_h WuT[h]^T @ attnT[h] + corr + bu

Degenerate softmax rows (all keys masked / no causal-visible key) are fixed
up exactly via two per-head mean-of-V vectors folded through the output
projection as a rank-2 correction (host-computed 0/1 row selectors).
Matmuls run as float32r (full PE rate at N>=256, near-fp32 precision).
"""

import numpy as np
import ml_dtypes

import concourse.bacc as bacc
import concourse.mybir as mybir
import concourse.tile as tile
from concourse.bass_utils import run_bass_kernel_spmd

F32 = mybir.dt.float32
F32R = mybir.dt.float32r
BF16 = mybir.dt.bfloat16
F16 = mybir.dt.float16

import os
SCORE_DT = {"f32r": F32R, "f16": F16}[os.environ.get("K_SCORE_DT", "f32r")]
SOFT_DT = {"f32r": F32R, "f16": F16}[os.environ.get("K_SOFT_DT", "f16")]
TRI_NEG = -60000.0 if SOFT_DT == F16 else -1.0e30
_SOFT_NP = {F16: "float16", F32R: "float32"}

B, TQ, TK, E, H = 8, 1024, 1024, 128, 8
HE = H * E
SCALE = float(E) ** -0.5
NEG = -1.0e30


def _chunks(kc):
    """Absolute column ranges for score row kc, split at the 512 PSUM bank."""
    lo = kc * 128
    if lo < 512:
        return [(lo, 512), (512, TQ)]
    return [(lo, TQ)]


def _build():
    nc = bacc.Bacc("TRN2", target_bir_lowering=False, debug=False)
    dp = nc.declare_dram_parameter
    d_qT = dp("qT", [E, TQ], F32R, isOutput=False)
    d_kT = dp("kT", [E, TK], F32R, isOutput=False)
    d_wqT = dp("wqT", [E, HE], F32R, isOutput=False)
    d_wkT = dp("wkT", [E, HE], F32R, isOutput=False)
    d_wvT = dp("wvT", [E, HE], F32R, isOutput=False)
    d_wuT = dp("wuT", [HE, E], SOFT_DT, isOutput=False)
    d_mkb = dp("mkbias", [128, 8], F32, isOutput=False)
    d_tri = dp("trineg", [128, 128], SOFT_DT, isOutput=False)
    d_idb = dp("identb", [128, 128], SOFT_DT, isOutput=False)
    d_case = dp("caserow", [1, TQ], F32R, isOutput=False)
    d_brow = dp("brows", [2, TQ], F32R, isOutput=False)
    d_w2 = dp("w2", [2, E], F32R, isOutput=False)
    d_ones1 = dp("onesk1", [1, 128], F32R, isOutput=False)
    d_ones = dp("ones128", [128, 128], SOFT_DT, isOutput=False)
    d_bu = dp("bu", [E, 1], F32, isOutput=False)
    d_out = dp("out", [E, TQ], F32, isOutput=True)

    Exp = mybir.ActivationFunctionType.Exp
    Ident = mybir.ActivationFunctionType.Identity
    mult = mybir.AluOpType.mult
    mm = nc.tensor.matmul

    with tile.TileContext(nc) as tc:
        with (
            tc.tile_pool(name="const", bufs=1) as cp,
            tc.tile_pool(name="persist", bufs=1) as pp,
        ):
            # ---- constants ----
            wu = []
            for h in range(H):
                t = cp.tile([128, 128], SOFT_DT, tag=f"wu{h}", name=f"wu{h}")
                nc.sync.dma_start(out=t[:], in_=d_wuT[h * 128 : (h + 1) * 128, :])
                wu.append(t)
            mkb = cp.tile([128, 8], F32, tag="mkb", name="mkb")
            nc.sync.dma_start(out=mkb[:], in_=d_mkb[:])
            tri = cp.tile([128, 128], SOFT_DT, tag="tri", name="tri")
            nc.sync.dma_start(out=tri[:], in_=d_tri[:])
            idb = cp.tile([128, 128], SOFT_DT, tag="idb", name="idb")
            nc.sync.dma_start(out=idb[:], in_=d_idb[:])
            case = cp.tile([1, TQ], F32R, tag="case", name="case")
            nc.sync.dma_start(out=case[:], in_=d_case[:])
            brow = cp.tile([2, TQ], F32R, tag="brow", name="brow")
            nc.sync.dma_start(out=brow[:], in_=d_brow[:])
            ones1 = cp.tile([1, 128], F32R, tag="ones1", name="ones1")
            nc.sync.dma_start(out=ones1[:], in_=d_ones1[:])
            ones = cp.tile([128, 128], SOFT_DT, tag="ones", name="ones")
            nc.sync.dma_start(out=ones[:], in_=d_ones[:])
            bu = cp.tile([E, 1], F32, tag="bu", name="bu")
            nc.sync.dma_start(out=bu[:], in_=d_bu[:])
            w2 = cp.tile([2, 128], F32R, tag="w2", name="w2")
            nc.sync.dma_start(out=w2[:], in_=d_w2[:])

            # ---- persistent activations ----
            QT = [pp.tile([128, TQ], SCORE_DT, tag=f"QT{h}", name=f"QT{h}") for h in range(H)]
            KT = [pp.tile([128, TK], SCORE_DT, tag=f"KT{h}", name=f"KT{h}") for h in range(H)]
            V = [pp.tile([128, HE], SOFT_DT, tag=f"V{kc}", name=f"V{kc}") for kc in range(8)]
            attnT = [
                pp.tile([128, TQ], SOFT_DT, tag=f"attnT{h}", name=f"attnT{h}")
                for h in range(H)
            ]

            # ---- phase 1: projections ----
            with (
                tc.tile_pool(name="proj", bufs=1) as jp,
                tc.tile_pool(name="ppsum", bufs=3, space="PSUM") as jps,
            ):
                qTs = jp.tile([E, TQ], F32R, tag="qTs", name="qTs")
                nc.sync.dma_start(out=qTs[:], in_=d_qT[:])
                kTs = jp.tile([E, TK], F32R, tag="kTs", name="kTs")
                nc.sync.dma_start(out=kTs[:], in_=d_kT[:])
                wq = jp.tile([E, HE], F32R, tag="wq", name="wq")
                nc.sync.dma_start(out=wq[:], in_=d_wqT[:])
                wk = jp.tile([E, HE], F32R, tag="wk", name="wk")
                nc.sync.dma_start(out=wk[:], in_=d_wkT[:])
                wv = jp.tile([E, HE], F32R, tag="wv", name="wv")
                nc.sync.dma_start(out=wv[:], in_=d_wvT[:])

                n_evac = 0

                def evac(dst, src):
                    nonlocal n_evac
                    if n_evac % 2 == 0:
                        nc.vector.tensor_copy(dst, src)
                    else:
                        nc.scalar.copy(dst, src)
                    n_evac += 1

                for h in range(H):
                    ps = jps.tile([128, TQ], F32, tag="pps", name=f"psq{h}")
                    for a, b in ((0, 512), (512, TQ)):
                        mm(ps[:, a:b], wq[:, h * 128 : (h + 1) * 128],
                           qTs[:, a:b], start=True, stop=True)
                    evac(QT[h][:], ps[:])
                for h in range(H):
                    ps = jps.tile([128, TK], F32, tag="pps", name=f"psk{h}")
                    for a, b in ((0, 512), (512, TK)):
                        mm(ps[:, a:b], wk[:, h * 128 : (h + 1) * 128],
                           kTs[:, a:b], start=True, stop=True)
                    evac(KT[h][:], ps[:])
                for kc in range(8):
                    ps = jps.tile([128, HE], F32, tag="pps", name=f"psv{kc}")
                    for a, b in ((0, 512), (512, HE)):
                        mm(ps[:, a:b], kTs[:, kc * 128 : (kc + 1) * 128],
                           wv[:, a:b], start=True, stop=True)
                    evac(V[kc][:], ps[:])


            # ---- phase 3: attention, unit = (head, 512-col query half) ----
            with (
                tc.tile_pool(name="stps", bufs=3, space="PSUM") as sp,
                tc.tile_pool(name="accps", bufs=2, space="PSUM") as ap_,
                tc.tile_pool(name="atp", bufs=5) as atp,
                tc.tile_pool(name="ssp", bufs=2) as ssp,
            ):
                units = [(h, half) for h in range(H) for half in (0, 1)]
                state = {}

                def emit_epilogue(u):
                    h, half = units[u]
                    q0 = half * 512
                    sum_ps, out_ps = state[u]
                    sum_sb = ssp.tile([128, 512], F32, tag="sum_sb",
                                      name=f"ssb{u}")
                    nc.scalar.copy(sum_sb[:], sum_ps[:])
                    rb = ssp.tile([128, 512], F32, tag="rb", name=f"rb{u}")
                    nc.vector.reciprocal(out=rb[:], in_=sum_sb[:])
                    nc.vector.tensor_tensor(
                        out=attnT[h][:, q0 : q0 + 512], in0=out_ps[:],
                        in1=rb[:], op=mult,
                    )

                for u, (h, half) in enumerate(units):
                    q0 = half * 512
                    kcs = range(4) if half == 0 else range(8)
                    klast = kcs[-1]
                    sum_ps = ap_.tile([128, 512], F32, tag="sum_ps", name=f"sum{u}")
                    out_ps = ap_.tile([128, 512], F32, tag="out_ps", name=f"out{u}")
                    state[u] = (sum_ps, out_ps)
                    ats = {}

                    def consume(kc):
                        r0 = max(kc * 128 - q0, 0)
                        n = 512 - r0
                        mm(sum_ps[:, r0:512], ones[:], ats[kc][:, 0:n],
                           start=(kc == 0), stop=False)
                        mm(out_ps[:, r0:512],
                           V[kc][:, h * 128 : (h + 1) * 128],
                           ats[kc][:, 0:n], start=(kc == 0),
                           stop=(kc == klast))

                    for kc in kcs:
                        r0 = max(kc * 128 - q0, 0)
                        n = 512 - r0
                        diag = q0 <= kc * 128 < q0 + 512
                        st = sp.tile([128, 512], F32, tag="st", name=f"st{u}_{kc}")
                        mm(st[:, r0:512], KT[h][:, kc * 128 : (kc + 1) * 128],
                           QT[h][:, q0 + r0 : q0 + 512], start=True,
                           stop=not diag)
                        if diag:
                            mm(st[:, r0 : r0 + 128], idb[:], tri[:],
                               start=False, stop=True)
                        at = atp.tile([128, 512], SOFT_DT, tag="at", name=f"at{u}_{kc}")
                        ats[kc] = at
                        nc.scalar.activation(
                            out=at[:, 0:n], in_=st[:, r0:512], func=Exp,
                            bias=mkb[:, kc : kc + 1], scale=SCALE,
                        )
                        if kc >= 2:
                            consume(kc - 2)
                        if kc == 2 and u >= 1:
                            emit_epilogue(u - 1)
                    if klast >= 1:
                        consume(klast - 1)
                    if klast == 0 and u >= 1:
                        emit_epilogue(u - 1)
                    consume(klast)
                    mm(sum_ps[:], ones1[:], case[:, q0 : q0 + 512],
                       start=False, stop=True)
                emit_epilogue(len(units) - 1)

            # ---- phase 4: output projection ----
            with tc.tile_pool(name="finps", bufs=1, space="PSUM") as fp:
                fin = fp.tile([128, TQ], F32, tag="fin", name="fin")
                for h in range(H):
                    for a, b in ((0, 512), (512, TQ)):
                        mm(fin[:, a:b], wu[h][:], attnT[h][:, a:b],
                           start=(h == 0), stop=False)
                for a, b in ((0, 512), (512, TQ)):
                    mm(fin[:, a:b], w2[:], brow[:, a:b],
                       start=False, stop=True)
                outsb = pp.tile([E, TQ], F32, tag="outsb", name="outsb")
                nc.scalar.activation(
                    out=outsb[:], in_=fin[:], func=Ident, bias=bu[:, 0:1], scale=1.0
                )
                nc.sync.dma_start(out=d_out[:], in_=outsb[:])

    nc.compile()
    return nc


_NC = None


def _get_nc():
    global _NC
    if _NC is None:
        _NC = _build()
    return _NC


def _host_prep(q, k, mask_q, mask_k, Wq, Wk, Wv, Wu, bu):
    shared = {
        "wqT": np.ascontiguousarray(Wq.T),
        "wkT": np.ascontiguousarray(Wk.T),
        "wvT": np.ascontiguousarray(Wv.T),
        "wuT": np.ascontiguousarray(Wu.T).astype(_SOFT_NP[SOFT_DT]),
        "trineg": (TRI_NEG * np.tril(np.ones((128, 128), np.float32), -1)).astype(_SOFT_NP[SOFT_DT]),
        "identb": np.eye(128).astype(_SOFT_NP[SOFT_DT]),
        "onesk1": np.ones((1, 128), np.float32),
        "ones128": np.ones((128, 128)).astype(_SOFT_NP[SOFT_DT]),
        "bu": np.ascontiguousarray(bu[:, None]),
    }
    WuWv = (Wu @ Wv).astype(np.float32)
    in_maps = []
    for b in range(B):
        mq = mask_q[b, :, 0].astype(np.float32)
        mk = mask_k[b, :, 0].astype(np.float32)
        c01 = (np.cumsum(mk) >= 1.0).astype(np.float32)
        caseA = mq * c01
        b1 = mq * (1.0 - c01)
        b2 = 1.0 - mq
        s1m = 1.0 - mk
        denom = max(float(s1m.sum()), 1.0)
        wvecs = np.stack([s1m / denom, np.full(TK, 1.0 / TK, np.float32)], axis=1)
        w2 = (wvecs.T.astype(np.float32) @ k[b]) @ WuWv.T
        m = dict(shared)
        m["qT"] = np.ascontiguousarray(q[b].T)
        m["kT"] = np.ascontiguousarray(k[b].T)
        m["mkbias"] = np.ascontiguousarray(
            ((mk - 1.0) * -NEG).reshape(8, 128).T
        ).astype(np.float32)
        m["caserow"] = ((1.0 - caseA) * -NEG)[None, :].astype(np.float32)
        m["brows"] = np.stack([b1, b2]).astype(np.float32)
        m["w2"] = np.ascontiguousarray(w2.astype(np.float32))
        in_maps.append(m)
    return in_maps


def kernel(q, k, mask_q, mask_k, Wq, Wk, Wv, Wu, bu):
    nc = _get_nc()
    in_maps = _host_prep(q, k, mask_q, mask_k, Wq, Wk, Wv, Wu, bu)
    res = run_bass_kernel_spmd(nc, in_maps, list(range(B)))
    out = np.stack([np.ascontiguousarray(res.results[b]["out"].T) for b in range(B)])
    return out.astype(np.float32)
